# revision 20
# baseline (speedup 1.0000x reference)
"""Trainium2 Bass kernel for nn_ARCDModelPTADisc (GNN message passing).

Sharding (8 cores):
  - Domain stage (512 nodes): replicated.
  - Skill stage (2048 nodes): output-sharded 256 skills/core (attention
    epilogue is elementwise-heavy), AllGather between.
  - Question stack (16384): row-sharded 2048 q/core; BN stats via tiny
    AllReduce; backward SpMM partial-summed via AllReduce.
  - Student stage (8192): A_uq column-sharded by question shard;
    ReduceScatter; LN on local 1024-student shard.

Layout convention on device: feature matrices live "feature-major"
(transposed, [256, n]); feature matmuls H@W.T consume feature-major H as
the stationary operand and produce node-major WH, which is exactly the
stationary operand the SpMM needs; the SpMM emits feature-major output.
No on-device transposes anywhere.  Degree normalizations are folded into
the adjacency matrices on the host; biases enter via K=1 seed matmuls
(node-major) or per-partition activation bias (feature-major).
"""

import os
import sys

for _p in ("/opt/trn_rl_repo", "/root/.axon_site/_ro/trn_rl_repo"):
    if os.path.isdir(_p) and _p not in sys.path:
        sys.path.insert(0, _p)

import numpy as np
from contextlib import ExitStack

import concourse.bass as bass
import concourse.bacc as bacc
import concourse.mybir as mybir
from concourse import tile
from concourse.bass_utils import run_bass_kernel_spmd

F32 = mybir.dt.float32
BF16 = mybir.dt.bfloat16
MMDT = BF16          # dtype of matmul operands (accumulate is always f32)
MMNP = np.float32 if MMDT == F32 else np.dtype("bfloat16") if hasattr(np, "bfloat16") else None

D = 256
DIN = 128
ND = 512          # domains
NS = 2048         # skills
NQ = 16384        # questions
NU = 8192         # users
H = 4
DK = 64
P = 8             # cores
SK_SH = NS // P   # 256 skills per core
Q_SH = NQ // P    # 2048 questions per core
U_SH = NU // P    # 1024 users per core
EPS = 1e-5
RG = [list(range(P))]

# frow bias indices (node-major featmm seeds)
IB_DB, IB_D2S, IB_SB, IB_QF0, IB_QB0, IB_QF1, IB_QB1, IB_SU = range(8)
NB = 8
# fcol per-feature column indices
(IC_DPROJ, IC_SPROJ, IC_DLNG, IC_DLNB, IC_SLNG, IC_SLNB, IC_MLNG, IC_MLNB,
 IC_ULNG, IC_ULNB, IC_BNG0, IC_BNB0, IC_BNG1, IC_BNB1) = range(14)
NC = 14

AF = mybir.ActivationFunctionType
ALU = mybir.AluOpType


def _to_mm(x):
    """Cast to the matmul dtype on host (via ml_dtypes bf16 if needed)."""
    if MMDT == F32:
        return np.ascontiguousarray(x, dtype=np.float32)
    import ml_dtypes
    return np.ascontiguousarray(x.astype(ml_dtypes.bfloat16))


def _basic_norm_T(A):
    """Return An.T for _basic's normalized adjacency (fp32)."""
    N = A.shape[0]
    At = A + np.eye(N, dtype=A.dtype)
    Dg = At.sum(1)
    Dinv = np.where(Dg > 0, Dg ** -0.5, 0.0).astype(np.float32)
    An = Dinv[:, None] * At * Dinv[None, :]
    return np.ascontiguousarray(An.T)


def _al_mat(a):
    """Block-diagonal [256, 4] matrix M with M[h*64+k, h] = a[h, k]."""
    M = np.zeros((D, H), np.float32)
    for h in range(H):
        M[h * DK:(h + 1) * DK, h] = a[h]
    return M


def host_prep(H_s, H_d, A_dom, A_ds, A_pre, A_qs, A_uq, params):
    """Build the per-core input maps (all numpy)."""
    p = params
    f32 = np.float32

    # ---- replicated smalls ----
    rep = {}
    rep["HdT"] = _to_mm(H_d.T)                       # [128, 512]
    rep["HsT"] = _to_mm(H_s.T)                       # [128, 2048]
    for nm, w in [("dbWT", p["db_W"]), ("dprojWT", p["d_proj_W"]),
                  ("sbWT", p["sb_W"]), ("sprojWT", p["s_proj_W"])]:
        rep[nm] = _to_mm(w.T)                        # [128, 256]
    for nm, w in [("daWT", p["da_W"]), ("saWT", p["sa_W"]),
                  ("d2sWT", p["d2s_W"]), ("suWT", p["su_W"]),
                  ("qfWT0", p["qf0_W"]), ("qbWT0", p["qb0_W"]),
                  ("qfWT1", p["qf1_W"]), ("qbWT1", p["qb1_W"])]:
        rep[nm] = _to_mm(np.asarray(w.T).reshape(2, 128, 256))

    frow = np.zeros((1, NB * D), f32)
    for i, b in enumerate([p["db_b"], p["d2s_b"], p["sb_b"], p["qf0_b"],
                           p["qb0_b"], p["qf1_b"], p["qb1_b"], p["su_b"]]):
        frow[0, i * D:(i + 1) * D] = b
    rep["frow"] = _to_mm(frow)

    fcol = np.zeros((D, NC), f32)
    for i, v in enumerate([p["d_proj_b"], p["s_proj_b"], p["d_ln_g"],
                           p["d_ln_b"], p["s_ln_g"], p["s_ln_b"],
                           p["m_ln_g"], p["m_ln_b"], p["u_ln_g"],
                           p["u_ln_b"], p["bn0_g"], p["bn0_b"],
                           p["bn1_g"], p["bn1_b"]]):
        fcol[:, i] = v
    rep["fcol"] = np.ascontiguousarray(fcol.reshape(2, 128, NC))

    rep["ALd"] = _to_mm(_al_mat(np.asarray(p["da_al"])).reshape(2, 128, H))
    rep["ALs"] = _to_mm(_al_mat(np.asarray(p["sa_al"])).reshape(2, 128, H))
    rep["ARrep_d"] = np.broadcast_to(
        np.asarray(p["da_ar"], f32)[None], (128, H, DK)).copy()
    rep["ARrep_s"] = np.broadcast_to(
        np.asarray(p["sa_ar"], f32)[None], (128, H, DK)).copy()

    AnDomT = _basic_norm_T(np.asarray(A_dom, f32))
    rep["AnDomT"] = _to_mm(AnDomT.reshape(4, 128, ND))
    maskT_dom = ((A_dom + np.eye(ND, dtype=f32)) > 0).astype(f32).T
    rep["maskT_dom"] = _to_mm(maskT_dom.reshape(4, 128, ND))

    AnPreT = _basic_norm_T(np.asarray(A_pre, f32))       # [2048, 2048] (An.T)
    maskP = ((A_pre + np.eye(NS, dtype=f32)) > 0).astype(f32)

    deg_ds = np.clip(np.asarray(A_ds, f32).sum(1, keepdims=True), 1.0, None)
    AdsT = (np.asarray(A_ds, f32) / deg_ds).T            # [512, 2048]

    A_qs = np.asarray(A_qs, f32)
    A_uq = np.asarray(A_uq, f32)
    deg_q = np.clip(A_qs.sum(1, keepdims=True), 1.0, None)     # [NQ,1]
    deg_s = np.clip(A_qs.sum(0, keepdims=True), 1.0, None)     # [1,NS]
    deg_u = np.clip(A_uq.sum(1, keepdims=True), 1.0, None)     # [NU,1]

    q_embT = np.asarray(p["q_emb"], f32).T               # [256, NQ]
    u_embT = np.asarray(p["u_emb"], f32).T               # [256, NU]

    in_maps = []
    for r in range(P):
        m = dict(rep)
        sk = slice(r * SK_SH, (r + 1) * SK_SH)
        qq = slice(r * Q_SH, (r + 1) * Q_SH)
        uu = slice(r * U_SH, (r + 1) * U_SH)
        m["HsT_my"] = _to_mm(H_s.T[:, sk])                       # [128,256]
        m["AnPreT_my"] = _to_mm(AnPreT[:, sk].reshape(16, 128, SK_SH))
        m["maskT_pre_my"] = _to_mm(maskP[sk].T.reshape(16, 128, SK_SH))
        m["AdsT_my"] = _to_mm(AdsT[:, sk].reshape(4, 128, SK_SH))
        # fwd: rhs = (A_qs/deg_q).T[:, my_q]  [2048s, 2048q]
        m["AqsT_n"] = _to_mm(
            (A_qs[qq] / deg_q[qq]).T.reshape(16, 128, Q_SH))
        # bwd: rhs = (A_qs/deg_s)[my_q, :]    [2048q, 2048s]
        m["Aqs_cs"] = _to_mm(
            (A_qs[qq] / deg_s).reshape(16, 128, NS))
        # student: rhs = (A_uq/deg_u).T[my_q, :]  [2048q, 8192u]
        m["AuqT"] = _to_mm(
            (A_uq[:, qq] / deg_u).T.reshape(16, 128, NU))
        m["qembT"] = np.ascontiguousarray(q_embT[:, qq].reshape(2, 128, Q_SH))
        m["uembT"] = np.ascontiguousarray(u_embT[:, uu].reshape(2, 128, U_SH))
        in_maps.append(m)
    return in_maps


# ---------------------------------------------------------------------------
# device program
# ---------------------------------------------------------------------------

def build_nc():
    nc = bacc.Bacc("TRN2", target_bir_lowering=False, debug=False,
                   num_devices=P)

    def din(name, shape, dt=MMDT):
        return nc.dram_tensor(name, list(shape), dt, kind="ExternalInput")

    t_in = {}
    t_in["HdT"] = din("HdT", (128, ND))
    t_in["HsT"] = din("HsT", (128, NS))
    t_in["HsT_my"] = din("HsT_my", (128, SK_SH))
    for nm in ("dbWT", "dprojWT", "sbWT", "sprojWT"):
        t_in[nm] = din(nm, (128, 256))
    for nm in ("daWT", "saWT", "d2sWT", "suWT", "qfWT0", "qbWT0", "qfWT1",
               "qbWT1"):
        t_in[nm] = din(nm, (2, 128, 256))
    t_in["frow"] = din("frow", (1, NB * D))
    t_in["fcol"] = din("fcol", (2, 128, NC), F32)
    t_in["ALd"] = din("ALd", (2, 128, H))
    t_in["ALs"] = din("ALs", (2, 128, H))
    t_in["ARrep_d"] = din("ARrep_d", (128, H, DK), F32)
    t_in["ARrep_s"] = din("ARrep_s", (128, H, DK), F32)
    t_in["AnDomT"] = din("AnDomT", (4, 128, ND))
    t_in["maskT_dom"] = din("maskT_dom", (4, 128, ND))
    t_in["AnPreT_my"] = din("AnPreT_my", (16, 128, SK_SH))
    t_in["maskT_pre_my"] = din("maskT_pre_my", (16, 128, SK_SH))
    t_in["AdsT_my"] = din("AdsT_my", (4, 128, SK_SH))
    t_in["AqsT_n"] = din("AqsT_n", (16, 128, Q_SH))
    t_in["Aqs_cs"] = din("Aqs_cs", (16, 128, NS))
    t_in["AuqT"] = din("AuqT", (16, 128, NU))
    t_in["qembT"] = din("qembT", (2, 128, Q_SH), F32)
    t_in["uembT"] = din("uembT", (2, 128, U_SH), F32)

    out_hsT = nc.dram_tensor("out_hsT", [2, 128, SK_SH], F32,
                             kind="ExternalOutput")
    out_hqT = nc.dram_tensor("out_hqT", [2, 128, Q_SH], F32,
                             kind="ExternalOutput")
    out_huT = nc.dram_tensor("out_huT", [2, 128, U_SH], F32,
                             kind="ExternalOutput")

    with ExitStack() as ctx:
        tc = ctx.enter_context(tile.TileContext(nc))
        cst = ctx.enter_context(tc.tile_pool(name="cst", bufs=1))
        big = ctx.enter_context(tc.tile_pool(name="big", bufs=1))
        sb = ctx.enter_context(tc.tile_pool(name="sb", bufs=2))
        pp = ctx.enter_context(tc.tile_pool(name="pp", bufs=1, space="PSUM"))
        dram = ctx.enter_context(tc.tile_pool(name="dram", bufs=1,
                                              space="DRAM"))

        dma = nc.sync.dma_start

        def P_mm(name, shape=(128, 512), bufs=3, tag="mm"):
            return pp.tile(list(shape), F32, name=name, tag=tag, bufs=bufs)

        # ---- constants / resident inputs ----
        ones_mm = cst.tile([128, 128], MMDT, name="ones_mm")
        nc.vector.memset(ones_mm[:], 1.0)
        ones_f = cst.tile([128, 128], F32, name="ones_f")
        nc.vector.memset(ones_f[:], 1.0)
        eps_col = cst.tile([128, 1], F32, name="eps_col")
        nc.vector.memset(eps_col[:], EPS)

        def load(name, dt=MMDT):
            """Load a DRAM input into SBUF.  3D inputs [k,128,n] become a
            list of k tiles [128, n] (partition dim must be first)."""
            shp = list(t_in[name].shape)
            if len(shp) == 2 or shp[0] == 128:
                t = cst.tile(shp, dt, name=f"sb_{name}")
                dma(t[:], t_in[name][:])
                return t
            k = shp[0]
            outs = []
            for i in range(k):
                t = cst.tile(shp[1:], dt, name=f"sb_{name}_{i}")
                dma(t[:], t_in[name][i])
                outs.append(t)
            return outs

        HdT = load("HdT")
        HsT = load("HsT")
        HsT_my = load("HsT_my")
        dbWT = load("dbWT"); dprojWT = load("dprojWT")
        sbWT = load("sbWT"); sprojWT = load("sprojWT")
        W2 = {nm: load(nm) for nm in ("daWT", "saWT", "d2sWT", "suWT",
                                      "qfWT0", "qbWT0", "qfWT1", "qbWT1")}
        frow = load("frow")
        fcol = load("fcol", F32)
        ALd = load("ALd"); ALs = load("ALs")
        ARrep_d = load("ARrep_d", F32)
        ARrep_s = load("ARrep_s", F32)
        AnDomT = load("AnDomT")
        maskT_dom = load("maskT_dom")
        AnPreT = load("AnPreT_my")
        maskT_pre = load("maskT_pre_my")
        AdsT = load("AdsT_my")

        def fc(t, idx):
            return fcol[t][:, idx:idx + 1]

        # ------------------------------------------------------------------
        def featmm_nm(lhsT_tiles, wt, bias_idx, n_m, tag):
            """Node-major WH = H@W.T (+b).  lhsT_tiles: feature-major H
            (k tiles [128, n]); wt: [128,256] or [2,128,256] tile.
            Returns n_m tiles [128, 256] MMDT."""
            outs = []
            nk = len(lhsT_tiles)
            for m in range(n_m):
                ps = P_mm(f"ps_{tag}_{m}")
                if bias_idx is not None:
                    nc.tensor.matmul(
                        ps[:, 0:256], ones_mm[0:1, 0:128],
                        frow[0:1, bias_idx * D:(bias_idx + 1) * D],
                        start=True, stop=False)
                for k in range(nk):
                    nc.tensor.matmul(
                        ps[:, 0:256],
                        lhsT_tiles[k][:, m * 128:(m + 1) * 128], wt[k][:],
                        start=(bias_idx is None and k == 0),
                        stop=(k == nk - 1))
                o = sb.tile([128, 256], MMDT, name=f"wh_{tag}_{m}",
                            tag="whnm", bufs=16)
                nc.vector.tensor_copy(o[:], ps[:, 0:256])
                outs.append(o)
            return outs

        def ln_fm(x_tiles, n, gi, bi, out_tiles, out_mm=None):
            """LayerNorm over features, feature-major [2][128, n] fp32."""
            for j in range(0, n, 512):
                w = min(512, n - j)
                mps = pp.tile([1, 512], F32, name="ln_m", tag="den", bufs=2)
                for f in range(2):
                    nc.tensor.matmul(mps[0:1, 0:w], ones_f[:, 0:1],
                                     x_tiles[f][:, j:j + w],
                                     start=(f == 0), stop=(f == 1))
                eps_ = pp.tile([1, 512], F32, name="ln_e", tag="den", bufs=2)
                for f in range(2):
                    sq = sb.tile([128, 512], F32, name="ln_sq", tag="lnsq",
                                 bufs=1)
                    nc.scalar.activation(sq[:, 0:w], x_tiles[f][:, j:j + w],
                                         AF.Square)
                    nc.tensor.matmul(eps_[0:1, 0:w], ones_f[:, 0:1],
                                     sq[:, 0:w], start=(f == 0), stop=(f == 1))
                m_sb = sb.tile([1, 512], F32, name="ln_msb", tag="lnrow",
                               bufs=4)
                nc.scalar.activation(m_sb[0:1, 0:w], mps[0:1, 0:w],
                                     AF.Copy, scale=1.0 / D)
                e_sb = sb.tile([1, 512], F32, name="ln_esb", tag="lnrow",
                               bufs=4)
                nc.scalar.activation(e_sb[0:1, 0:w], eps_[0:1, 0:w],
                                     AF.Copy, scale=1.0 / D)
                v_sb = sb.tile([1, 512], F32, name="ln_vsb", tag="lnrow",
                               bufs=4)
                nc.vector.tensor_mul(v_sb[0:1, 0:w], m_sb[0:1, 0:w],
                                     m_sb[0:1, 0:w])
                nc.vector.tensor_sub(v_sb[0:1, 0:w], e_sb[0:1, 0:w],
                                     v_sb[0:1, 0:w])
                nc.scalar.activation(v_sb[0:1, 0:w], v_sb[0:1, 0:w],
                                     AF.Sqrt, bias=eps_col[0:1, :])
                r_sb = sb.tile([1, 512], F32, name="ln_rsb", tag="lnrow",
                               bufs=4)
                nc.vector.reciprocal(r_sb[0:1, 0:w], v_sb[0:1, 0:w])
                mb = P_mm("ln_mb")
                nc.tensor.matmul(mb[:, 0:w], ones_f[0:1, 0:128],
                                 m_sb[0:1, 0:w], start=True, stop=True)
                rb = pp.tile([128, 512], F32, name="ln_rb", tag="rbc", bufs=1)
                nc.tensor.matmul(rb[:, 0:w], ones_f[0:1, 0:128],
                                 r_sb[0:1, 0:w], start=True, stop=True)
                for f in range(2):
                    t = sb.tile([128, 512], F32, name="ln_t", tag="lnt",
                                bufs=1)
                    nc.vector.tensor_sub(t[:, 0:w], x_tiles[f][:, j:j + w],
                                         mb[:, 0:w])
                    nc.vector.tensor_mul(t[:, 0:w], t[:, 0:w], rb[:, 0:w])
                    nc.vector.tensor_scalar(out_tiles[f][:, j:j + w],
                                            t[:, 0:w], fc(f, gi), fc(f, bi),
                                            ALU.mult, ALU.add)
                    if out_mm is not None:
                        nc.vector.tensor_copy(out_mm[f][:, j:j + w],
                                              out_tiles[f][:, j:j + w])

        def elu_fm(x_tiles, n):
            for f in range(2):
                r = sb.tile([128, n], F32, name="elu_r", tag="elur", bufs=2)
                nc.scalar.activation(r[:], x_tiles[f][:], AF.Relu)
                d_ = sb.tile([128, n], F32, name="elu_d", tag="elud", bufs=2)
                nc.vector.tensor_sub(d_[:], x_tiles[f][:], r[:])
                nc.scalar.activation(d_[:], d_[:], AF.Exp)
                nc.vector.tensor_add(x_tiles[f][:], r[:], d_[:])
                nc.vector.tensor_scalar_add(x_tiles[f][:], x_tiles[f][:],
                                            -1.0)

        def attention(n, nm_tiles_k, feat_my_k, WTs, AL, ARrep, maskT,
                      attnT, tag):
            """GAT attention.
            n: my output count (free dim); nm_tiles_k: feature-major FULL
            input (k=2 tiles [128, n_all]); feat_my_k: feature-major MY
            slice (k=2 tiles [128, n]); WTs: [2,128,256]; maskT: [n_all/128]
            tiles [128, n] f32; attnT: out f32 tiles [2][128, n]."""
            n_all = nm_tiles_k[0].shape[-1]
            n_m = n_all // 128
            # Wh node-major (full); srN computed inline from a transient
            # f32 copy of each psum tile
            Wh = []
            srN = []
            for m in range(n_m):
                ps = P_mm(f"at_wh_{tag}_{m}")
                for k in range(2):
                    nc.tensor.matmul(ps[:, 0:256],
                                     nm_tiles_k[k][:, m * 128:(m + 1) * 128],
                                     WTs[k], start=(k == 0), stop=(k == 1))
                o = sb.tile([128, 256], MMDT, name=f"at_whmm_{tag}_{m}",
                            tag="atwh", bufs=16)
                nc.vector.tensor_copy(o[:], ps[:, 0:256])
                Wh.append(o)
                tmp = sb.tile([128, H, DK], F32, name=f"at_sr3_{tag}_{m}",
                              tag="sr3", bufs=2)
                nc.vector.tensor_mul(
                    tmp[:],
                    ps[:, 0:256].rearrange("p (h k) -> p h k", h=H),
                    ARrep[:])
                so = sb.tile([128, H], F32, name=f"at_srn_{tag}_{m}",
                             tag=f"srn_{tag}", bufs=n_m)
                nc.vector.tensor_reduce(so[:], tmp[:], mybir.AxisListType.X,
                                        ALU.add)
                srN.append(so)
            # WhT for my columns ([2][128, n])
            WhT = []
            for f in range(2):
                ps = P_mm(f"at_wt_{tag}_{f}")
                for k in range(2):
                    nc.tensor.matmul(ps[:, 0:n],
                                     WTs[k][:, f * 128:(f + 1) * 128],
                                     feat_my_k[k][:, 0:n],
                                     start=(k == 0), stop=(k == 1))
                o = sb.tile([128, n], MMDT, name=f"at_wtmm_{tag}_{f}",
                            tag=f"atwt_{tag}", bufs=2)
                nc.vector.tensor_copy(o[:], ps[:, 0:n])
                WhT.append(o)
            # slT: per-head row [1, n] at partition 0 (matmul rhs needs
            # base partition 0), packed at cols h*512 of one tile
            slT = sb.tile([1, H * 512], MMDT, name=f"at_slT_{tag}",
                          tag="slT", bufs=2)
            for h in range(H):
                sps = pp.tile([1, 512], F32, name=f"at_sl_{tag}_{h}",
                              tag="den", bufs=2)
                for k in range(2):
                    nc.tensor.matmul(sps[0:1, 0:n], AL[k][:, h:h + 1],
                                     WhT[k][:, 0:n],
                                     start=(k == 0), stop=(k == 1))
                nc.vector.tensor_copy(slT[0:1, h * 512:h * 512 + n],
                                      sps[0:1, 0:n])
            # per-head masked softmax + einsum
            for h in range(H):
                ops_ = pp.tile([64, 512], F32, name=f"at_o_{tag}_{h}",
                               tag="attno", bufs=2)
                dps = pp.tile([1, 512], F32, name=f"at_d_{tag}_{h}",
                              tag="den", bufs=2)
                for m in range(n_m):
                    eps_ = P_mm(f"at_e_{tag}_{h}_{m}")
                    nc.tensor.matmul(eps_[:, 0:n], ones_mm[0:1, 0:128],
                                     slT[0:1, h * 512:h * 512 + n],
                                     start=True, stop=True)
                    t1 = sb.tile([128, 512], F32, name="at_t1", tag="at_t1",
                                 bufs=2)
                    nc.scalar.activation(t1[:, 0:n], eps_[:, 0:n], AF.Lrelu,
                                         bias=srN[m][:, h:h + 1], alpha=0.2)
                    nc.scalar.activation(t1[:, 0:n], t1[:, 0:n], AF.Exp)
                    emm = sb.tile([128, 512], MMDT, name="at_emm",
                                  tag="at_emm", bufs=3)
                    nc.vector.tensor_mul(emm[:, 0:n], t1[:, 0:n],
                                         maskT[m][:, 0:n])
                    nc.tensor.matmul(dps[0:1, 0:n], ones_mm[:, 0:1],
                                     emm[:, 0:n], start=(m == 0),
                                     stop=(m == n_m - 1))
                    nc.tensor.matmul(ops_[0:64, 0:n],
                                     Wh[m][:, h * DK:(h + 1) * DK],
                                     emm[:, 0:n], start=(m == 0),
                                     stop=(m == n_m - 1))
                dsb = sb.tile([1, 512], F32, name="at_dsb", tag="at_dsb",
                              bufs=2)
                nc.vector.tensor_copy(dsb[0:1, 0:n], dps[0:1, 0:n])
                rec = sb.tile([1, 512], F32, name="at_rec", tag="at_dsb",
                              bufs=2)
                nc.vector.reciprocal(rec[0:1, 0:n], dsb[0:1, 0:n])
                rmm = sb.tile([1, 512], MMDT, name="at_rmm", tag="at_rmm",
                              bufs=2)
                nc.vector.tensor_copy(rmm[0:1, 0:n], rec[0:1, 0:n])
                rps = pp.tile([64, 512], F32, name="at_rb", tag="rbc", bufs=1)
                nc.tensor.matmul(rps[0:64, 0:n], ones_mm[0:1, 0:64],
                                 rmm[0:1, 0:n], start=True, stop=True)
                rbs = sb.tile([64, 512], F32, name="at_rbs", tag="at_rbs",
                              bufs=2)
                nc.vector.tensor_copy(rbs[0:64, 0:n], rps[0:64, 0:n])
                f, o = h // 2, (h % 2) * DK
                nc.vector.tensor_mul(attnT[f][o:o + DK, 0:n],
                                     ops_[0:64, 0:n], rbs[0:64, 0:n])
            elu_fm(attnT, n)

        # ==================================================================
        # Stage A: domain (replicated, n=512)
        # ==================================================================
        WHd = featmm_nm([HdT], [dbWT], IB_DB, 4, "whd")
        hd0T = [big.tile([128, ND], MMDT, name=f"hd0T_{f}") for f in range(2)]
        for f in range(2):
            ps = P_mm(f"ps_hd0_{f}")
            for m in range(4):
                nc.tensor.matmul(ps[:, 0:ND],
                                 WHd[m][:, f * 128:(f + 1) * 128],
                                 AnDomT[m], start=(m == 0), stop=(m == 3))
            nc.scalar.activation(hd0T[f][:], ps[:, 0:ND], AF.Relu)
        attnTD = [big.tile([128, ND], F32, name=f"attnTD_{f}")
                  for f in range(2)]
        attention(ND, hd0T, hd0T, W2["daWT"], ALd, ARrep_d,
                  maskT_dom, attnTD, "dom")
        # hd = LN(attn + Hd@dprojWT + b)
        xd = attnTD
        for f in range(2):
            ps = P_mm(f"ps_dproj_{f}")
            nc.tensor.matmul(ps[:, 0:ND], dprojWT[:, f * 128:(f + 1) * 128],
                             HdT[:], start=True, stop=True)
            t = sb.tile([128, ND], F32, name="dproj_t", tag="lnt", bufs=1)
            nc.scalar.activation(t[:], ps[:, 0:ND], AF.Copy)
            nc.vector.tensor_scalar_add(t[:], t[:], fc(f, IC_DPROJ))
            nc.vector.tensor_add(xd[f][:], xd[f][:], t[:])
        hdT_mm = [big.tile([128, ND], MMDT, name=f"hdTm_{f}")
                  for f in range(2)]
        ln_fm(xd, ND, IC_DLNG, IC_DLNB, xd, hdT_mm)
        # h_d2s (my 256 skill columns)
        WHds = featmm_nm(hdT_mm, W2["d2sWT"], IB_D2S, 4, "whds")
        d2sT = [big.tile([128, SK_SH], F32, name=f"d2sT_{f}")
                for f in range(2)]
        for f in range(2):
            ps = P_mm(f"ps_d2s_{f}")
            for m in range(4):
                nc.tensor.matmul(ps[:, 0:SK_SH],
                                 WHds[m][:, f * 128:(f + 1) * 128],
                                 AdsT[m], start=(m == 0), stop=(m == 3))
            nc.scalar.activation(d2sT[f][:], ps[:, 0:SK_SH], AF.Relu)

        # ==================================================================
        # Stage B: skills (sharded, my 256 columns)
        # ==================================================================
        WHs = featmm_nm([HsT], [sbWT], IB_SB, 16, "whs")
        hs0T_my = [big.tile([128, SK_SH], MMDT, name=f"hs0T_{f}")
                   for f in range(2)]
        for f in range(2):
            ps = P_mm(f"ps_hs0_{f}")
            for m in range(16):
                nc.tensor.matmul(ps[:, 0:SK_SH],
                                 WHs[m][:, f * 128:(f + 1) * 128],
                                 AnPreT[m], start=(m == 0), stop=(m == 15))
            nc.scalar.activation(hs0T_my[f][:], ps[:, 0:SK_SH], AF.Relu)
        # AllGather hs0
        ag_in = dram.tile([2, 128, SK_SH], MMDT, name="ag_hs0_in")
        ag_out = dram.tile([P, 2, 128, SK_SH], MMDT, name="ag_hs0_out",
                           addr_space="Shared")
        for f in range(2):
            dma(ag_in[f], hs0T_my[f][:])
        nc.gpsimd.collective_compute("AllGather", ALU.bypass,
                                     replica_groups=RG,
                                     ins=[ag_in.opt()], outs=[ag_out.opt()])
        hs0T_full = [big.tile([128, NS], MMDT, name=f"hs0F_{f}",
                              tag="fullT", bufs=2) for f in range(2)]
        for r in range(P):
            for f in range(2):
                dma(hs0T_full[f][:, r * SK_SH:(r + 1) * SK_SH], ag_out[r, f])
        attnTS = [big.tile([128, SK_SH], F32, name=f"attnTS_{f}")
                  for f in range(2)]
        attention(SK_SH, hs0T_full, hs0T_my, W2["saWT"], ALs, ARrep_s,
                  maskT_pre, attnTS, "sk")
        xs = attnTS
        for f in range(2):
            ps = P_mm(f"ps_sproj_{f}")
            nc.tensor.matmul(ps[:, 0:SK_SH],
                             sprojWT[:, f * 128:(f + 1) * 128],
                             HsT_my[:], start=True, stop=True)
            t = sb.tile([128, SK_SH], F32, name="sproj_t", tag="lnt", bufs=1)
            nc.scalar.activation(t[:], ps[:, 0:SK_SH], AF.Copy)
            nc.vector.tensor_scalar_add(t[:], t[:], fc(f, IC_SPROJ))
            nc.vector.tensor_add(xs[f][:], xs[f][:], t[:])
        hskT = [big.tile([128, SK_SH], F32, name=f"hskT_{f}")
                for f in range(2)]
        ln_fm(xs, SK_SH, IC_SLNG, IC_SLNB, hskT)
        for f in range(2):
            nc.vector.tensor_add(hskT[f][:], hskT[f][:], d2sT[f][:])
        hsoT_mm = [big.tile([128, SK_SH], MMDT, name=f"hsoTm_{f}")
                   for f in range(2)]
        ln_fm(hskT, SK_SH, IC_MLNG, IC_MLNB, hskT, hsoT_mm)
        for f in range(2):
            dma(out_hsT[f], hskT[f][:])
        # AllGather h_s_out
        ag2_in = dram.tile([2, 128, SK_SH], MMDT, name="ag_hso_in")
        ag2_out = dram.tile([P, 2, 128, SK_SH], MMDT, name="ag_hso_out",
                            addr_space="Shared")
        for f in range(2):
            dma(ag2_in[f], hsoT_mm[f][:])
        nc.gpsimd.collective_compute("AllGather", ALU.bypass,
                                     replica_groups=RG,
                                     ins=[ag2_in.opt()], outs=[ag2_out.opt()])
        hsrcT = [big.tile([128, NS], MMDT, name=f"hsrcT_{f}",
                          tag="fullT", bufs=2) for f in range(2)]
        for r in range(P):
            for f in range(2):
                dma(hsrcT[f][:, r * SK_SH:(r + 1) * SK_SH], ag2_out[r, f])

        # ==================================================================
        # Stage C: question stack (sharded, my 2048 rows)
        # ==================================================================
        HtT = [big.tile([128, Q_SH], F32, name=f"HtT_{f}") for f in range(2)]
        HtT_mm = [big.tile([128, Q_SH], MMDT, name=f"HtTm_{f}")
                  for f in range(2)]
        for f in range(2):
            dma(HtT[f][:], t_in["qembT"][f])
        for li in range(2):
            qfW = W2[f"qfWT{li}"]
            qbW = W2[f"qbWT{li}"]
            ib_f = IB_QF0 if li == 0 else IB_QF1
            ib_b = IB_QB0 if li == 0 else IB_QB1
            ic_g = IC_BNG0 if li == 0 else IC_BNG1
            ic_b = IC_BNB0 if li == 0 else IC_BNB1
            WHf = featmm_nm(hsrcT, qfW, ib_f, 16, f"whf{li}")
            bns = big.tile([128, 4], F32, name=f"bns_{li}")
            nc.vector.memset(bns[:], 0.0)
            for j in range(4):
                ats = []
                for m in range(16):
                    at = sb.tile([128, 512], MMDT, name=f"aqs_{li}_{j}_{m}",
                                 tag="a_str", bufs=18)
                    dma(at[:], t_in["AqsT_n"].ap()[m, :, j * 512:(j + 1) * 512])
                    ats.append(at)
                for f in range(2):
                    ps = P_mm(f"ps_qf_{li}_{j}_{f}")
                    for m in range(16):
                        nc.tensor.matmul(ps[:],
                                         WHf[m][:, f * 128:(f + 1) * 128],
                                         ats[m][:], start=(m == 0),
                                         stop=(m == 15))
                    t = sb.tile([128, 512], F32, name="qf_t", tag="qf_t",
                                bufs=3)
                    nc.scalar.activation(t[:], ps[:], AF.Relu)
                    xv = HtT[f][:, j * 512:(j + 1) * 512]
                    nc.vector.tensor_add(xv, t[:], xv)
                    rt = sb.tile([128, 1], F32, name="bn_rt", tag="bn_rt",
                                 bufs=4)
                    nc.vector.tensor_reduce(rt[:], xv, mybir.AxisListType.X,
                                            ALU.add)
                    nc.vector.tensor_add(bns[:, 2 * f:2 * f + 1],
                                         bns[:, 2 * f:2 * f + 1], rt[:])
                    sq = sb.tile([128, 512], F32, name="bn_sq", tag="qf_t",
                                 bufs=3)
                    nc.scalar.activation(sq[:], xv, AF.Square)
                    rt2 = sb.tile([128, 1], F32, name="bn_rt2", tag="bn_rt",
                                  bufs=4)
                    nc.vector.tensor_reduce(rt2[:], sq[:],
                                            mybir.AxisListType.X, ALU.add)
                    nc.vector.tensor_add(bns[:, 2 * f + 1:2 * f + 2],
                                         bns[:, 2 * f + 1:2 * f + 2], rt2[:])
            # BN stats AllReduce
            bn_in = dram.tile([128, 4], F32, name=f"bn_in_{li}")
            bn_out = dram.tile([128, 4], F32, name=f"bn_out_{li}",
                               addr_space="Shared")
            dma(bn_in[:], bns[:])
            nc.gpsimd.collective_compute("AllReduce", ALU.add,
                                         replica_groups=RG,
                                         ins=[bn_in.opt()],
                                         outs=[bn_out.opt()])
            bnr = sb.tile([128, 4], F32, name=f"bnr_{li}", tag="bnr", bufs=2)
            dma(bnr[:], bn_out[:])
            for f in range(2):
                mean = sb.tile([128, 1], F32, name="bn_mean", tag="bn_st",
                               bufs=8)
                nc.scalar.activation(mean[:], bnr[:, 2 * f:2 * f + 1],
                                     AF.Copy, scale=1.0 / NQ)
                ex2 = sb.tile([128, 1], F32, name="bn_ex2", tag="bn_st",
                              bufs=8)
                nc.scalar.activation(ex2[:], bnr[:, 2 * f + 1:2 * f + 2],
                                     AF.Copy, scale=1.0 / NQ)
                var = sb.tile([128, 1], F32, name="bn_var", tag="bn_st",
                              bufs=8)
                nc.vector.tensor_mul(var[:], mean[:], mean[:])
                nc.vector.tensor_sub(var[:], ex2[:], var[:])
                nc.scalar.activation(var[:], var[:], AF.Sqrt,
                                     bias=eps_col[:])
                rstd = sb.tile([128, 1], F32, name="bn_rstd", tag="bn_st",
                               bufs=8)
                nc.vector.reciprocal(rstd[:], var[:])
                scale = sb.tile([128, 1], F32, name="bn_scale", tag="bn_st",
                                bufs=8)
                nc.vector.tensor_mul(scale[:], rstd[:], fc(f, ic_g))
                shift = sb.tile([128, 1], F32, name="bn_shift", tag="bn_st",
                                bufs=8)
                nc.vector.tensor_mul(shift[:], mean[:], scale[:])
                nc.vector.tensor_sub(shift[:], fc(f, ic_b), shift[:])
                nc.vector.tensor_scalar(HtT[f][:], HtT[f][:], scale[:],
                                        shift[:], ALU.mult, ALU.add)
                nc.vector.tensor_copy(HtT_mm[f][:], HtT[f][:])
            # backward bipartite (dead after the last layer: H_src
            # is never read again -- the reference computes it but h_q
            # only depends on H_tgt)
            if li == 1:
                continue
            WHb = featmm_nm(HtT_mm, qbW, ib_b, 16, f"whb{li}")
            bp_in = dram.tile([2, 128, NS], F32, name=f"bp_in_{li}")
            bp_out = dram.tile([2, 128, NS], F32, name=f"bp_out_{li}",
                               addr_space="Shared")
            for j in range(4):
                acs = []
                for m in range(16):
                    at = sb.tile([128, 512], MMDT, name=f"acs_{li}_{j}_{m}",
                                 tag="a_str", bufs=18)
                    dma(at[:], t_in["Aqs_cs"].ap()[m, :, j * 512:(j + 1) * 512])
                    acs.append(at)
                for f in range(2):
                    ps = P_mm(f"ps_qb_{li}_{j}_{f}")
                    for m in range(16):
                        nc.tensor.matmul(ps[:],
                                         WHb[m][:, f * 128:(f + 1) * 128],
                                         acs[m][:], start=(m == 0),
                                         stop=(m == 15))
                    t = sb.tile([128, 512], F32, name="qb_t", tag="qf_t",
                                bufs=3)
                    nc.vector.tensor_copy(t[:], ps[:])
                    dma(bp_in[f, :, j * 512:(j + 1) * 512], t[:])
            nc.gpsimd.collective_compute("AllReduce", ALU.add,
                                         replica_groups=RG,
                                         ins=[bp_in.opt()],
                                         outs=[bp_out.opt()])
            hsrcT = [big.tile([128, NS], MMDT, name=f"hsrc{li}_{f}",
                              tag="fullT", bufs=2) for f in range(2)]
            for f in range(2):
                for j in range(4):
                    srt = sb.tile([128, 512], F32, name="srt", tag="srt",
                                  bufs=2)
                    dma(srt[:], bp_out[f, :, j * 512:(j + 1) * 512])
                    nc.scalar.activation(
                        hsrcT[f][:, j * 512:(j + 1) * 512], srt[:], AF.Relu)
        for f in range(2):
            dma(out_hqT[f], HtT[f][:])

        # ==================================================================
        # Stage D: students
        # ==================================================================
        WHu = featmm_nm(HtT_mm, W2["suWT"], IB_SU, 16, "whu")
        rs_in = dram.tile([P, 2, 128, U_SH], F32, name="rs_in")
        rs_out = dram.tile([2, 128, U_SH], F32, name="rs_out")
        for j in range(16):
            rank, off = j // 2, (j % 2) * 512
            aus = []
            for m in range(16):
                at = sb.tile([128, 512], MMDT, name=f"auq_{j}_{m}",
                             tag="a_str", bufs=18)
                dma(at[:], t_in["AuqT"].ap()[m, :, j * 512:(j + 1) * 512])
                aus.append(at)
            for f in range(2):
                ps = P_mm(f"ps_uq_{j}_{f}")
                for m in range(16):
                    nc.tensor.matmul(ps[:], WHu[m][:, f * 128:(f + 1) * 128],
                                     aus[m][:], start=(m == 0), stop=(m == 15))
                t = sb.tile([128, 512], F32, name="uq_t", tag="qf_t", bufs=3)
                nc.vector.tensor_copy(t[:], ps[:])
                dma(rs_in[rank, f, :, off:off + 512], t[:])
        nc.gpsimd.collective_compute("ReduceScatter", ALU.add,
                                     replica_groups=RG,
                                     ins=[rs_in.opt()], outs=[rs_out.opt()])
        xu = [big.tile([128, U_SH], F32, name=f"xu_{f}") for f in range(2)]
        for f in range(2):
            ut = sb.tile([128, U_SH], F32, name="ut", tag="ut", bufs=1)
            dma(ut[:], rs_out[f])
            nc.scalar.activation(ut[:], ut[:], AF.Relu)
            ue = sb.tile([128, U_SH], F32, name="ue", tag="ue", bufs=1)
            dma(ue[:], t_in["uembT"][f])
            nc.vector.tensor_add(xu[f][:], ut[:], ue[:])
        ln_fm(xu, U_SH, IC_ULNG, IC_ULNB, xu)
        for f in range(2):
            dma(out_huT[f], xu[f][:])

    nc.compile()
    return nc


_NC_CACHE = {}


def kernel(H_s, H_d, A_dom, A_ds, A_pre, A_qs, A_uq, params):
    in_maps = host_prep(np.asarray(H_s, np.float32), np.asarray(H_d, np.float32),
                        np.asarray(A_dom, np.float32), np.asarray(A_ds, np.float32),
                        np.asarray(A_pre, np.float32), np.asarray(A_qs, np.float32),
                        np.asarray(A_uq, np.float32),
                        {k: np.asarray(v, np.float32) for k, v in params.items()})
    if "nc" not in _NC_CACHE:
        _NC_CACHE["nc"] = build_nc()
    nc = _NC_CACHE["nc"]
    res = run_bass_kernel_spmd(nc, in_maps, list(range(P))).results
    # assemble
    hs_parts = [res[r]["out_hsT"].reshape(D, SK_SH) for r in range(P)]
    h_s = np.concatenate(hs_parts, axis=1).T
    hq_parts = [res[r]["out_hqT"].reshape(D, Q_SH) for r in range(P)]
    h_q = np.concatenate(hq_parts, axis=1).T
    hu_parts = [res[r]["out_huT"].reshape(D, U_SH) for r in range(P)]
    h_u = np.concatenate(hu_parts, axis=1).T
    return (np.ascontiguousarray(h_s, np.float32),
            np.ascontiguousarray(h_q, np.float32),
            np.ascontiguousarray(h_u, np.float32))
